# revision 33
# baseline (speedup 1.0000x reference)
"""Trainium2 Bass kernel for nn_Block_65841848648273 (spiking transformer block).

Sharding: data-parallel over B (16 -> 2 per core x 8 cores), sync-BN via
AllReduce of per-channel (sum, sumsq). Activations kept column-major
[C, T*B_loc*N] so BN stats are per-partition free-dim reductions and LIF
runs on free-dim slices. v is computed row-major [R, C] directly (swapped
matmul operands) for the attention o = attn @ v contraction.

All dense fp32 GEMMs (q/k/v/proj/fc1 and attn@v) run in float32r: the PE
streams fp32 operands at bf16 row rate (1 cyc/row for free dim >= 256)
with ~2^-13 relative rounding — measured 1.5e-4 matmul rel err on HW,
well inside the 2e-2 gate. Spikes are exact 0/1 in bf16, so attention
q@k^T runs at bf16 rate and fc2 runs as two bf16 passes against
hi/lo-split weights with the h spikes kept resident in SBUF (no DRAM
spill round-trip).

Engine schedule: matmul phases are emitted k -> v -> q so the v-stats
AllReduce and its affine-coefficient chain hide under the q GEMM + q LIF;
the attention QK stage runs one block ahead of AV; fc1 pipelines 3 y1
chunks deep; fc2 accumulates kt-outer so it overlaps the fc1 tail. LIF
state updates (w += 2^t z, hard reset w *= (s < 0.5)) run as fused
scalar_tensor_tensor ops on the otherwise-idle Pool engine, interleaved
across ctiles so no engine stalls on one chain.

LIF rescaling: v_t = (v_{t-1} + z_t)/2  ==>  w_t = w_{t-1} + 2^t z_t with
w_t = 2^{t+1} v_t; spike iff w_t >= 2^{t+1}; hard reset on spike.
Linear biases are dropped entirely: BN(y + b) == BN(y) exactly.
The SSA 0.125 scale is folded into the v-layer BN affine on the host.
"""

import numpy as np
import ml_dtypes

import concourse.bass as bass
import concourse.mybir as mybir
import concourse.tile as tile
from concourse import bacc
from concourse.bass import ts, ds

F32 = mybir.dt.float32
F32R = mybir.dt.float32r
BF16 = mybir.dt.bfloat16
ALU = mybir.AluOpType
ACTF = mybir.ActivationFunctionType

T, B, N, C, H, HID = 4, 16, 256, 512, 8, 2048
NCORES = 8
BL = B // NCORES            # 2 batches per core
R = T * BL * N              # 2048 rows per core
TS_ = BL * N                # 512 rows per timestep
DH = C // H                 # 64
EPS = 1e-5
SCALE = 0.125
NTOT = T * B * N            # 16384 global BN count

CT_C = C // 128             # 4
CT_H = HID // 128            # 16
RC = R // 512               # 4 row chunks
RT = R // 128               # 16 row tiles

_BUILD_CACHE = {}


def build_nc(single_core=False):
    key = "nc1" if single_core else "nc"
    if key in _BUILD_CACHE:
        return _BUILD_CACHE[key]
    nc = bacc.Bacc("TRN2", target_bir_lowering=False, debug=False,
                   enable_asserts=True,
                   num_devices=1 if single_core else NCORES)

    tens = {}
    tens["xT"] = nc.dram_tensor("xT", [C, R], F32R, kind="ExternalInput")
    for nm, shp, dt_ in (("wqT", [C, C], F32R), ("wkT", [C, C], F32R),
                         ("wvT", [C, C], F32R), ("wpT", [C, C], F32R),
                         ("wf1T", [C, HID], F32R),
                         ("wf2T_hi", [HID, C], BF16), ("wf2T_lo", [HID, C], BF16)):
        tens[nm] = nc.dram_tensor(nm, shp, dt_, kind="ExternalInput")
    tens["onesr"] = nc.dram_tensor("onesr", [128, 1], F32R, kind="ExternalInput")
    tens["onesrw"] = nc.dram_tensor("onesrw", [2, 128], F32R, kind="ExternalInput")
    for nm, nct in (("gq", CT_C), ("bq", CT_C), ("gk", CT_C), ("bk", CT_C),
                    ("gp", CT_C), ("bp", CT_C), ("gf1", CT_H), ("bf1", CT_H),
                    ("gf2", CT_C), ("bf2", CT_C)):
        tens[nm] = nc.dram_tensor(nm, [128, nct], F32, kind="ExternalInput")
    # v-layer gamma/beta as [1, C] rows, SCALE folded
    tens["gv"] = nc.dram_tensor("gv", [1, C], F32, kind="ExternalInput")
    tens["bv"] = nc.dram_tensor("bv", [1, C], F32, kind="ExternalInput")
    tens["outT"] = nc.dram_tensor("outT", [C, R], F32R, kind="ExternalOutput")

    with tile.TileContext(nc, pool_alloc_mode="queue") as tc:
        _emit(nc, tc, tens, single_core=single_core)
    nc.compile()
    _BUILD_CACHE[key] = nc
    return nc


def _emit(nc, tc, tens, single_core=False):
    groups = [list(range(NCORES))]

    def all_reduce(arin, arout):
        # DRAM in/out: SBUF collectives are broken on current HW/runtime
        if single_core:
            # timing-sim variant: no collectives; a DRAM->DRAM copy via the
            # SP DMA queue stands in (real AR latency is modeled separately)
            nc.sync.dma_start(out=arout[:], in_=arin[:])
        else:
            nc.gpsimd.collective_compute(
                "AllReduce", ALU.add, replica_groups=groups,
                ins=[arin.opt()], outs=[arout.opt()])

    # ---------- long-lived pools ----------
    small = tc.alloc_tile_pool(name="small", bufs=1)
    stats = tc.alloc_tile_pool(name="stats", bufs=1)
    dram = tc.alloc_tile_pool(name="dram", bufs=1, space="DRAM")

    ones_col = small.tile([128, 1], F32R)
    nc.sync.dma_start(out=ones_col, in_=tens["onesr"].ap())
    ones_row = small.tile([2, 128], F32R)
    nc.sync.dma_start(out=ones_row, in_=tens["onesrw"].ap())
    gb = {}
    for nm in ("gq", "bq", "gk", "bk", "gp", "bp", "gf1", "bf1", "gf2", "bf2"):
        tl = small.tile(list(tens[nm].shape), F32, name=f"sb_{nm}")
        nc.sync.dma_start(out=tl, in_=tens[nm].ap())
        gb[nm] = tl

    def alloc_coeffs(name, nct):
        return {"a": small.tile([128, nct], F32, name=f"{name}_a"),
                "c": small.tile([128, nct], F32, name=f"{name}_c"),
                "at": small.tile([128, T, nct], F32, name=f"{name}_at"),
                "ct": small.tile([128, T, nct], F32, name=f"{name}_ct")}

    cf_k = alloc_coeffs("k", CT_C)
    cf_q = alloc_coeffs("q", CT_C)
    cf_p = alloc_coeffs("p", CT_C)
    cf_f1 = [alloc_coeffs(f"f1_{j}", 2) for j in range(8)]
    cf_f2 = alloc_coeffs("f2", CT_C)

    # ---------- weights + x ----------
    px = tc.alloc_tile_pool(name="px", bufs=1)
    x_sb = px.tile([128, CT_C, R], F32R)
    # one pool whose 2 slots rotate through all the 32KB/partition tensors
    pyy = tc.alloc_tile_pool(name="pyy", bufs=2)
    pw = tc.alloc_tile_pool(name="pw", bufs=4)
    w_sb = {}
    for nm in ("wkT", "wvT", "wqT", "wpT"):
        w_sb[nm] = pw.tile([128, CT_C, C], F32R, name=f"sb_{nm}", tag="w")
    _xr = tens["xT"].ap().rearrange("(ci p) r -> p ci r", p=128)
    _wr = {nm: tens[nm].ap().rearrange("(ci p) co -> p ci co", p=128)
           for nm in ("wkT", "wqT", "wvT", "wpT")}
    # first matmul group needs wk + x[:, :, rc0]: land those first
    nc.sync.dma_start(out=w_sb["wkT"], in_=_wr["wkT"])
    for rc in range(RC):
        for ci in range(CT_C):
            eng = nc.sync if (ci % 2 == 0) else nc.scalar
            eng.dma_start(out=x_sb[:, ci, ts(rc, 512)],
                          in_=_xr[:, ci, ts(rc, 512)])
        if rc == 0:
            nc.sync.dma_start(out=w_sb["wvT"], in_=_wr["wvT"])
        elif rc == 1:
            nc.sync.dma_start(out=w_sb["wqT"], in_=_wr["wqT"])
        elif rc == 2:
            nc.sync.dma_start(out=w_sb["wpT"], in_=_wr["wpT"])
    pspk = tc.alloc_tile_pool(name="pspk", bufs=1)
    k_spk = pspk.tile([128, CT_C, R], BF16)
    q_spk = pspk.tile([128, CT_C, R], BF16)

    ps1 = tc.alloc_tile_pool(name="ps1", bufs=6, space="PSUM")

    # ================= helpers =================
    def mm_col(wtl, dst, rhs, nm, pspool, pstag, couts=range(CT_C),
               post_tile=None, post_co=None, evict_split=False):
        for co in couts:
            for rc in range(RC):
                ps = pspool.tile([128, 512], F32, tag=pstag,
                                 name=f"ps_{nm}_{co}_{rc}")
                for ci in range(CT_C):
                    nc.tensor.matmul(ps, wtl[:, ci, ts(co, 128)],
                                     rhs[:, ci, ts(rc, 512)],
                                     start=(ci == 0), stop=(ci == CT_C - 1))
                if evict_split and rc % 2 == 1:
                    nc.vector.tensor_copy(dst[:, co, ts(rc, 512)], ps)
                else:
                    nc.scalar.copy(dst[:, co, ts(rc, 512)], ps)
                if post_tile is not None:
                    post_tile(co, rc)
            if post_co is not None:
                post_co(co)

    def make_packer(nct, nm):
        """Interleavable bn_stats: bnst(ct, rc, y_rc) per evicted tile,
        aggr(ct) once a ctile is complete, finish() -> AllReduce handle."""
        bn = stats.tile([128, nct, RC, 6], F32, name=f"bn_{nm}", tag="bn6",
                        bufs=2)
        mv = stats.tile([128, nct, 2], F32, name=f"mv_{nm}", tag="mv")

        def bnst(ct, rc, y_rc):
            nc.vector.bn_stats(out=bn[:, ct, rc, :], in_=y_rc)

        def aggr(ct):
            nc.vector.bn_aggr(out=mv[:, ct, :], in_=bn[:, ct, :, :])

        def finish():
            pk = stats.tile([128, nct, 2], F32, name=f"pk_{nm}", tag="pk")
            m2 = stats.tile([128, nct], F32, name=f"m2_{nm}", tag="m2s")
            nc.vector.tensor_scalar(out=pk[:, :, 0:1], in0=mv[:, :, 0:1],
                                    scalar1=float(R), scalar2=None, op0=ALU.mult)
            nc.vector.tensor_tensor(out=m2, in0=mv[:, :, 0:1], in1=mv[:, :, 0:1],
                                    op=ALU.mult)
            nc.vector.tensor_tensor(out=m2, in0=mv[:, :, 1:2], in1=m2, op=ALU.add)
            nc.vector.tensor_scalar(out=pk[:, :, 1:2], in0=m2, scalar1=float(R),
                                    scalar2=None, op0=ALU.mult)
            arin = dram.tile([128, nct, 2], F32, name=f"arin_{nm}")
            arout = dram.tile([128, nct, 2], F32, name=f"arout_{nm}")
            nc.sync.dma_start(out=arin, in_=pk)
            all_reduce(arin, arout)
            return arout

        return bnst, aggr, finish

    def pack_hooks(dst, nct, nm):
        bnst, aggr, finish = make_packer(nct, nm)

        def post_tile(co, rc):
            bnst(co, rc, dst[:, co, ts(rc, 512)])

        def post_co(co):
            aggr(co)

        return post_tile, post_co, finish

    def coeffs(arout, nct, g_tl, b_tl, cfd, nm, at_off=0):
        """AR result -> a = g*rstd, c = beta - a*mean."""
        ar_sb = stats.tile([128, nct, 2], F32, name=f"ars_{nm}", tag="ars",
                           bufs=2)
        nc.sync.dma_start(out=ar_sb, in_=arout[:])
        mean = stats.tile([128, nct], F32, name=f"mea_{nm}", tag="mea")
        var = stats.tile([128, nct], F32, name=f"var_{nm}", tag="vars")
        m2 = stats.tile([128, nct], F32, name=f"m2c_{nm}", tag="m2c")
        nc.vector.tensor_scalar(out=mean, in0=ar_sb[:, :, 0:1], scalar1=1.0 / NTOT,
                                scalar2=None, op0=ALU.mult)
        nc.vector.tensor_scalar(out=var, in0=ar_sb[:, :, 1:2], scalar1=1.0 / NTOT,
                                scalar2=None, op0=ALU.mult)
        nc.vector.tensor_tensor(out=m2, in0=mean, in1=mean, op=ALU.mult)
        nc.vector.tensor_tensor(out=var, in0=var, in1=m2, op=ALU.subtract)
        # u = var + eps; r0 ~ rsqrt(u) via LUT sqrt + reciprocal, then one
        # Newton step r1 = r0*(1.5 - 0.5*u*r0^2) to reach fp32 accuracy
        # (ACT Sqrt LUT alone is ~7e-6 worst-case -> flips LIF spikes).
        u = stats.tile([128, nct], F32, name=f"u_{nm}", tag="rsu")
        nc.vector.tensor_scalar(out=u, in0=var, scalar1=EPS, scalar2=None,
                                op0=ALU.add)
        nc.scalar.activation(out=var, in_=u, func=ACTF.Sqrt)
        nc.vector.reciprocal(out=var, in_=var)          # r0
        nt = stats.tile([128, nct], F32, name=f"nt_{nm}", tag="rsn")
        nc.vector.tensor_tensor(out=nt, in0=u, in1=var, op=ALU.mult)
        nc.vector.tensor_tensor(out=nt, in0=nt, in1=var, op=ALU.mult)
        nc.vector.tensor_scalar(out=nt, in0=nt, scalar1=-0.5, scalar2=1.5,
                                op0=ALU.mult, op1=ALU.add)
        nc.vector.tensor_tensor(out=var, in0=var, in1=nt, op=ALU.mult)  # rstd
        a_dst = cfd["a"] if at_off == 0 and cfd["a"].shape[1] == nct \
            else cfd["a"][:, at_off:at_off + nct]
        c_dst = cfd["c"] if at_off == 0 and cfd["c"].shape[1] == nct \
            else cfd["c"][:, at_off:at_off + nct]
        nc.vector.tensor_tensor(out=a_dst, in0=g_tl, in1=var, op=ALU.mult)
        nc.vector.tensor_tensor(out=m2, in0=a_dst, in1=mean, op=ALU.mult)
        nc.vector.tensor_tensor(out=c_dst, in0=b_tl, in1=m2, op=ALU.subtract)
        for t in range(T):
            sc = float(2 ** t)
            nc.vector.tensor_scalar(out=cfd["at"][:, t, at_off:at_off + nct],
                                    in0=a_dst, scalar1=sc, scalar2=None,
                                    op0=ALU.mult)
            nc.vector.tensor_scalar(out=cfd["ct"][:, t, at_off:at_off + nct],
                                    in0=c_dst, scalar1=sc, scalar2=None,
                                    op0=ALU.mult)

    def lif_multi(items, nm, z_dve=False, add_dve=False, post_t=None):
        """Interleaved LIF over 2-ctile-wide groups: z = a*y + c per ctile
        on Act (or DVE two-scalar form when z_dve); the 2^t scale folds
        into a single wide Pool stt accumulate; reset is one fused wide
        stt w *= (s < 0.5); threshold compare is one wide DVE op.

        items: list of (ywide [128,G,R], cfd, c0, G, spk(t)->[128,G,TS_],
        extra(t, s_ap)|None).
        """
        ws = [stats.tile([128, it[3], TS_], F32, name=f"lw_{nm}_{i}",
                         tag="lifw", bufs=2) for i, it in enumerate(items)]
        for t in range(T):
            sl = ts(t, TS_)
            zs = []
            for i, (yw, cfd, c0, G, spk, extra) in enumerate(items):
                zdst = ws[i] if t == 0 else stats.tile(
                    [128, G, TS_], F32, name=f"lz_{nm}_{i}_{t}", tag="lifz",
                    bufs=2)
                for g in range(G):
                    cg = c0 + g
                    if z_dve:
                        nc.vector.tensor_scalar(out=zdst[:, g, :],
                                                in0=yw[:, g, sl],
                                                scalar1=cfd["at"][:, t, cg:cg + 1],
                                                scalar2=cfd["ct"][:, t, cg:cg + 1],
                                                op0=ALU.mult, op1=ALU.add)
                    else:
                        nc.scalar.activation(out=zdst[:, g, :], in_=yw[:, g, sl],
                                             func=ACTF.Identity,
                                             bias=cfd["ct"][:, t, cg:cg + 1],
                                             scale=cfd["at"][:, t, cg:cg + 1])
                zs.append(zdst)
            if t > 0:
                for i in range(len(items)):
                    if add_dve:
                        nc.vector.tensor_tensor(out=ws[i], in0=zs[i],
                                                in1=ws[i], op=ALU.add)
                    else:
                        nc.gpsimd.tensor_tensor(out=ws[i], in0=zs[i],
                                                in1=ws[i], op=ALU.add)
            s_aps = []
            for i, (yw, cfd, c0, G, spk, extra) in enumerate(items):
                s_ap = spk(t)
                nc.vector.tensor_scalar(out=s_ap, in0=ws[i],
                                        scalar1=float(2 ** (t + 1)),
                                        scalar2=None, op0=ALU.is_ge)
                s_aps.append(s_ap)
            for i, (yw, cfd, c0, G, spk, extra) in enumerate(items):
                if t < T - 1:
                    # hard reset w *= (s < 0.5): fused stt, DVE only (the
                    # Pool engine has no TensorScalar-class ISA support)
                    nc.vector.scalar_tensor_tensor(out=ws[i], in0=s_aps[i],
                                                   scalar=0.5, in1=ws[i],
                                                   op0=ALU.is_lt, op1=ALU.mult)
                if extra is not None:
                    extra(t, s_aps[i])
            if post_t is not None:
                post_t(t)

    # ============ phase 1+2: k GEMM -> v GEMM (+stats AR) -> q GEMM ======
    y_k = pyy.tile([128, CT_C, R], F32, tag="yy", name="y_k")
    pt_k, pc_k, fin_k = pack_hooks(y_k, CT_C, "k")
    mm_col(w_sb["wkT"], y_k, x_sb, "k", ps1, "mmps", post_tile=pt_k,
           post_co=pc_k, evict_split=True)
    ar_k = fin_k()

    # v row-major, float32r; channel sums ride the PE via ones-matmuls
    v_sb = pyy.tile([128, RT, C], F32R, tag="yy", name="v_sb")
    pvsq = tc.alloc_tile_pool(name="pvsq", bufs=1)
    for nm in ("gv", "bv"):
        tl = pvsq.tile(list(tens[nm].shape), F32, name=f"sb_{nm}")
        nc.sync.dma_start(out=tl, in_=tens[nm].ap())
        gb[nm] = tl
    psv = tc.alloc_tile_pool(name="psv", bufs=1, space="PSUM")
    vsum_ps = psv.tile([1, C], F32)
    vsq_ps = psv.tile([1, C], F32)
    for rt in range(RT):
        ps = ps1.tile([128, 512], F32, tag="mmps", name=f"ps_v_{rt}")
        for ci in range(CT_C):
            nc.tensor.matmul(ps, x_sb[:, ci, ts(rt, 128)], w_sb["wvT"][:, ci, :],
                             start=(ci == 0), stop=(ci == CT_C - 1))
        if rt % 2 == 0:
            nc.scalar.copy(v_sb[:, rt, :], ps)
        else:
            nc.vector.tensor_copy(v_sb[:, rt, :], ps)
        vsq = pvsq.tile([128, C], F32R, name=f"vsq_{rt}", tag="vsq", bufs=2)
        nc.vector.scalar_tensor_tensor(out=vsq, in0=v_sb[:, rt, :], scalar=1.0,
                                       in1=v_sb[:, rt, :], op0=ALU.mult,
                                       op1=ALU.mult)
        nc.tensor.matmul(vsum_ps, ones_col, v_sb[:, rt, :],
                         start=(rt == 0), stop=(rt == RT - 1), skip_group_check=True)
        nc.tensor.matmul(vsq_ps, ones_col, vsq,
                         start=(rt == 0), stop=(rt == RT - 1), skip_group_check=True)

    # v stats AllReduce kicked off immediately (hides under q GEMM + LIFs)
    arin_v = dram.tile([2, C], F32)
    arout_v = dram.tile([2, C], F32)
    vsum_sb1 = pvsq.tile([1, C], F32, name="vsum_sb1")
    vsq_sb1 = pvsq.tile([1, C], F32, name="vsq_sb1")
    nc.scalar.copy(vsum_sb1, vsum_ps)
    nc.scalar.copy(vsq_sb1, vsq_ps)
    nc.sync.dma_start(out=arin_v[0:1, :], in_=vsum_sb1)
    nc.sync.dma_start(out=arin_v[1:2, :], in_=vsq_sb1)
    all_reduce(arin_v, arout_v)

    coeffs(ar_k, CT_C, gb["gk"], gb["bk"], cf_k, "k")
    lif_multi([(y_k[:, 2 * i:2 * i + 2, :], cf_k, 2 * i, 2,
                (lambda t, i=i: k_spk[:, 2 * i:2 * i + 2, ts(t, TS_)]), None)
               for i in range(2)], "k")

    y_q = pyy.tile([128, CT_C, R], F32, tag="yy", name="y_q")
    pt_q, pc_q, fin_q = pack_hooks(y_q, CT_C, "q")
    mm_col(w_sb["wqT"], y_q, x_sb, "q", ps1, "mmps", post_tile=pt_q,
           post_co=pc_q, evict_split=True)
    ar_q = fin_q()

    # ---- v affine in [1, C] row layout ----
    # BN(v) = a*(v+c/a) with a folded into the proj weights (row scale), so
    # only one elementwise add per v tile remains.
    _arv = arout_v[:]
    nc.sync.dma_start(out=vsum_sb1, in_=bass.AP(
        tensor=_arv.tensor, offset=_arv.offset, ap=[[C, 1], [1, C]]))
    nc.sync.dma_start(out=vsq_sb1, in_=bass.AP(
        tensor=_arv.tensor, offset=_arv.offset + C, ap=[[C, 1], [1, C]]))
    vmean = pvsq.tile([1, C], F32, name="vmean")
    vvar = pvsq.tile([1, C], F32, name="vvar")
    vm2 = pvsq.tile([1, C], F32, name="vm2")
    va_row = vsum_sb1
    vc_row = vsq_sb1
    nc.vector.tensor_scalar(out=vmean, in0=vsum_sb1, scalar1=1.0 / NTOT,
                            scalar2=None, op0=ALU.mult)
    nc.vector.tensor_scalar(out=vvar, in0=vsq_sb1, scalar1=1.0 / NTOT,
                            scalar2=None, op0=ALU.mult)
    nc.vector.tensor_tensor(out=vm2, in0=vmean, in1=vmean, op=ALU.mult)
    nc.vector.tensor_tensor(out=vvar, in0=vvar, in1=vm2, op=ALU.subtract)
    vnt = pvsq.tile([1, C], F32, name="v_nt")
    nc.vector.tensor_scalar(out=vm2, in0=vvar, scalar1=EPS, scalar2=None,
                            op0=ALU.add)                       # vm2 = var+eps
    nc.scalar.activation(out=vvar, in_=vm2, func=ACTF.Sqrt)
    nc.vector.reciprocal(out=vvar, in_=vvar)
    nc.vector.tensor_tensor(out=vnt, in0=vm2, in1=vvar, op=ALU.mult)
    nc.vector.tensor_tensor(out=vnt, in0=vnt, in1=vvar, op=ALU.mult)
    nc.vector.tensor_scalar(out=vnt, in0=vnt, scalar1=-0.5, scalar2=1.5,
                            op0=ALU.mult, op1=ALU.add)
    nc.vector.tensor_tensor(out=vvar, in0=vvar, in1=vnt, op=ALU.mult)  # rstd
    nc.vector.tensor_tensor(out=va_row, in0=gb["gv"], in1=vvar, op=ALU.mult)
    nc.vector.tensor_tensor(out=vm2, in0=va_row, in1=vmean, op=ALU.mult)
    nc.vector.tensor_tensor(out=vc_row, in0=gb["bv"], in1=vm2, op=ALU.subtract)
    # cp = c/a; broadcast to 128 partitions via a K=1 PE matmul
    nc.vector.reciprocal(out=vnt, in_=va_row)
    cp_row = pvsq.tile([2, C], F32R, name="cp_row")
    nc.vector.memset(cp_row, 0.0)
    nc.vector.tensor_tensor(out=cp_row[0:1, :], in0=vc_row, in1=vnt,
                            op=ALU.mult)
    cp_bc = pspk.tile([128, C], F32, name="cp_bc")
    ps_bcc = ps1.tile([128, 512], F32, tag="mmps", name="ps_bcc")
    nc.tensor.matmul(ps_bcc, ones_row, cp_row, start=True, stop=True)
    nc.scalar.copy(cp_bc, ps_bcc)
    # scale the proj weight rows by a_d in place (a in [128, CT_C] ci-major)
    a_dram = dram.tile([1, C], F32, name="a_dram")
    nc.sync.dma_start(out=a_dram, in_=va_row)
    a_ci = stats.tile([128, CT_C], F32, name="a_ci")
    _ad = a_dram[:]
    nc.sync.dma_start(out=a_ci, in_=bass.AP(
        tensor=_ad.tensor, offset=_ad.offset, ap=[[1, 128], [128, CT_C]]))
    for ci in range(CT_C):
        nc.vector.tensor_scalar(out=w_sb["wpT"][:, ci, :],
                                in0=w_sb["wpT"][:, ci, :],
                                scalar1=a_ci[:, ci:ci + 1], scalar2=None,
                                op0=ALU.mult)

    coeffs(ar_q, CT_C, gb["gq"], gb["bq"], cf_q, "q")
    pvsq.release()
    psv.release()
    ps1.release()

    # ======= phase 3: attention, interleaved with the q LIF rounds =======
    # QK for blocks (2t, 2t+1) is emitted right after LIF round t produces
    # the t-slice of q spikes; AV trails one block behind its QK.
    oT = pyy.tile([128, CT_C, R], F32R, tag="yy", name="oT")
    ps_at = tc.alloc_tile_pool(name="ps_at", bufs=3, space="PSUM")
    ps_o = tc.alloc_tile_pool(name="ps_o", bufs=3, space="PSUM")
    p_at = tc.alloc_tile_pool(name="p_at", bufs=9)

    def emit_qk(tb):
        t, b = divmod(tb, BL)
        row0 = t * TS_ + b * N
        tiles = {}
        for h in range(H):
            hp = 64 * (h % 2)
            kq_p = slice(hp, hp + 64)
            hc = h // 2
            atp = ps_at.tile([128, 2, 256], F32, tag="at",
                             name=f"atp_{tb}_{h}")
            for mt in range(2):
                nc.tensor.matmul(atp[:, mt, :],
                                 k_spk[kq_p, hc, ds(row0 + mt * 128, 128)],
                                 q_spk[kq_p, hc, ds(row0, 256)],
                                 start=True, stop=True)
            at_sb = p_at.tile([128, 2, 256], F32R, tag="atsb",
                              name=f"at_{tb}_{h}")
            # split PSUM evictions across Act and DVE
            if h % 8 < 5:
                nc.scalar.copy(at_sb, atp)
            else:
                nc.vector.tensor_copy(at_sb, atp)
            tiles[h] = at_sb
        return tiles

    def emit_av(tb, tiles):
        t, b = divmod(tb, BL)
        row0 = t * TS_ + b * N
        rt0 = tb * 2
        for hp_i in range(H // 2):
            op = ps_o.tile([128, 256], F32, tag="o", name=f"op_{tb}_{hp_i}")
            for sub in range(2):
                h = hp_i * 2 + sub
                hp = 64 * sub
                for mt in range(2):
                    nc.tensor.matmul(op[hp:hp + 64, :],
                                     v_sb[:, rt0 + mt, ds(h * DH, DH)],
                                     tiles[h][:, mt, :],
                                     start=(mt == 0), stop=(mt == 1),
                                     tile_position=(0, hp))
            nc.scalar.copy(oT[:, hp_i, ds(row0, 256)], op)

    ats = {}

    def attn_post(t):
        # v + c/a for the row tiles this round's AV blocks consume (Pool)
        for rt in range(4 * t, 4 * t + 4):
            nc.gpsimd.tensor_tensor(out=v_sb[:, rt, :], in0=v_sb[:, rt, :],
                                    in1=cp_bc, op=ALU.add)
        for tb in (2 * t, 2 * t + 1):
            ats[tb] = emit_qk(tb)
            if tb - 1 in ats:
                emit_av(tb - 1, ats.pop(tb - 1))

    lif_multi([(y_q[:, 2 * i:2 * i + 2, :], cf_q, 2 * i, 2,
                (lambda t, i=i: q_spk[:, 2 * i:2 * i + 2, ts(t, TS_)]), None)
               for i in range(2)], "q", z_dve=True, post_t=attn_post)
    emit_av(T * BL - 1, ats.pop(T * BL - 1))

    ps_o.release()
    ps_at.release()
    p_at.release()
    pspk.release()

    # ================= phase 4: proj + residual (x2 in place on x_sb) =====
    ps2 = tc.alloc_tile_pool(name="ps2", bufs=4, space="PSUM")
    y_p = pyy.tile([128, CT_C, R], F32, tag="yy", name="y_p")
    pt_p, pc_p, fin_p = pack_hooks(y_p, CT_C, "p")
    mm_col(w_sb["wpT"], y_p, oT, "p", ps2, "mmps2", post_tile=pt_p,
           post_co=pc_p)

    # fc1 (f32r) weights loaded before the p-stats AllReduce is queued, so
    # the waiting AR copy can't block them on the SP DMA queue. The same
    # 8KB slots later rotate to the fc2 bf16 hi/lo weights.
    pw.release()
    pf = tc.alloc_tile_pool(name="pf", bufs=5, side="right")
    f1w = []
    for ci in range(CT_C):
        tw = pf.tile([128, HID], F32R, name=f"f1w_{ci}", tag="wbig")
        nc.sync.dma_start(out=tw, in_=tens["wf1T"].ap()[ts(ci, 128), :])
        f1w.append(tw)
    ar_p = fin_p()

    coeffs(ar_p, CT_C, gb["gp"], gb["bp"], cf_p, "p")
    for rc in range(RC):
        for ct in range(CT_C):
            z = stats.tile([128, 512], F32, name=f"pz_{ct}_{rc}", tag="lifz",
                           bufs=2)
            nc.vector.tensor_scalar(out=z, in0=y_p[:, ct, ts(rc, 512)],
                                    scalar1=cf_p["a"][:, ct:ct + 1],
                                    scalar2=cf_p["c"][:, ct:ct + 1],
                                    op0=ALU.mult, op1=ALU.add)
            nc.gpsimd.tensor_tensor(out=x_sb[:, ct, ts(rc, 512)],
                                    in0=x_sb[:, ct, ts(rc, 512)], in1=z,
                                    op=ALU.add)

    # ================= phase 5: fc1 + LIF (single float32r pass) ==========
    pyy.release()
    # h spikes stay resident in SBUF (64KB/partition, bf16 0/1 exact)
    ph_sb = tc.alloc_tile_pool(name="ph_sb", bufs=1, side="right")
    h_sb = ph_sb.tile([128, CT_H, R], BF16, name="h_sb")
    py1 = tc.alloc_tile_pool(name="py1", bufs=3, side="right")
    f1_chunks = [(c0, 2) for c0 in range(0, CT_H, 2)]
    for j, (co0, ncts) in enumerate(f1_chunks):
        y1 = py1.tile([128, ncts, R], F32, tag="yy1", name=f"y1_{j}")
        bnst, aggr, fin = make_packer(ncts, f"f1_{j}")
        for c4 in range(ncts):
            co = co0 + c4
            for rc in range(RC):
                ps = ps2.tile([128, 512], F32, tag="mmps2", name=f"ps_f1_{co}_{rc}")
                for ci in range(CT_C):
                    nc.tensor.matmul(ps, f1w[ci][:, ts(co, 128)],
                                     x_sb[:, ci, ts(rc, 512)],
                                     start=(ci == 0), stop=(ci == CT_C - 1))
                nc.scalar.copy(y1[:, c4, ts(rc, 512)], ps)
                bnst(c4, rc, y1[:, c4, ts(rc, 512)])
            aggr(c4)
        arout = fin()
        coeffs(arout, ncts, gb["gf1"][:, ds(co0, ncts)], gb["bf1"][:, ds(co0, ncts)],
               cf_f1[j], f"f1_{j}")
        lif_multi([(y1, cf_f1[j], 0, 2,
                    (lambda t, co0=co0: h_sb[:, co0:co0 + 2, ts(t, TS_)]),
                    None)], f"h{co0}")

    # fc2 bf16 hi/lo weights, split by co-half so the first fc2 quarter
    # only waits on a quarter of the load bytes
    f2w = {}
    for coh in range(2):
        for wi, src in enumerate(("wf2T_hi", "wf2T_lo")):
            tw = pf.tile([128, CT_H, 256], BF16, name=f"f2w_{wi}_{coh}",
                         tag="wbig")
            nc.sync.dma_start(
                out=tw,
                in_=tens[src].ap()[:, ts(coh, 256)].rearrange(
                    "(kt p) co -> p kt co", p=128))
            f2w[(wi, coh)] = tw

    # ================= phase 6: fc2 + LIF + out =================
    py1.release()
    ps2.release()
    py2 = tc.alloc_tile_pool(name="py2", bufs=2, side="right")
    ps3 = tc.alloc_tile_pool(name="ps3", bufs=8, space="PSUM")
    for half in range(2):
        cos = (half * 2, half * 2 + 1)
        y2 = py2.tile([128, 2, R], F32, tag="yy2", name=f"y2_{half}")
        bnst2, aggr2, fin2 = make_packer(2, f"f2_{half}")
        # kt-outer accumulation in 4-psum quarters: early-kt matmuls only
        # depend on early fc1 chunks, so fc2 overlaps the fc1 tail
        for rcp in range(2):
            rcs = (rcp * 2, rcp * 2 + 1)
            pss = {}
            for co in cos:
                for rc in rcs:
                    pss[(co, rc)] = ps3.tile([128, 512], F32, tag="mm3",
                                             name=f"ps_f2_{co}_{rc}")
            for kt in range(CT_H):
                for wi in range(2):
                    for co in cos:
                        wsb = f2w[(wi, co // 2)]
                        for rc in rcs:
                            nc.tensor.matmul(
                                pss[(co, rc)], wsb[:, kt, ts(co % 2, 128)],
                                h_sb[:, kt, ts(rc, 512)],
                                start=(kt == 0 and wi == 0),
                                stop=(kt == CT_H - 1 and wi == 1))
            for ic, co in enumerate(cos):
                for rc in rcs:
                    nc.scalar.copy(y2[:, ic, ts(rc, 512)], pss[(co, rc)])
                    bnst2(ic, rc, y2[:, ic, ts(rc, 512)])
                    if rcp == 1 and rc == rcs[-1]:
                        aggr2(ic)
        arout = fin2()
        coeffs(arout, 2, gb["gf2"][:, ts(half, 2)], gb["bf2"][:, ts(half, 2)],
               cf_f2, f"f2_{half}", at_off=half * 2)

        def spk_ap2(t):
            stile = stats.tile([128, 2, TS_], BF16, name=f"ms_{half}_{t}",
                               tag="mspk", bufs=1)
            return stile[:]

        def resadd(t, s_ap, c0=cos[0]):
            sl = ts(t, TS_)
            nc.vector.tensor_tensor(out=x_sb[:, c0:c0 + 2, sl],
                                    in0=x_sb[:, c0:c0 + 2, sl],
                                    in1=s_ap, op=ALU.add)

        lif_multi([(y2, cf_f2, half * 2, 2, spk_ap2, resadd)],
                  f"m{half}", z_dve=True, add_dve=True)
        for co in cos:
            nc.sync.dma_start(out=tens["outT"].ap()[ts(co, 128), :],
                              in_=x_sb[:, co, :])

    ps3.release()
    py2.release()
    ph_sb.release()
    pf.release()
    px.release()
    stats.release()
    small.release()
    dram.release()


# ===================== host side =====================

def _prep_shared(inp):
    bf = ml_dtypes.bfloat16
    f32 = np.float32
    d = {}
    d["wqT"] = np.ascontiguousarray(np.asarray(inp["qw"], f32).T)
    d["wkT"] = np.ascontiguousarray(np.asarray(inp["kw"], f32).T)
    d["wvT"] = np.ascontiguousarray(np.asarray(inp["vw"], f32).T)
    d["wpT"] = np.ascontiguousarray(np.asarray(inp["pw"], f32).T)
    d["wf1T"] = np.ascontiguousarray(np.asarray(inp["f1w"], f32).T)
    f2T = np.ascontiguousarray(np.asarray(inp["f2w"], f32).T)
    f2h = f2T.astype(bf)
    d["wf2T_hi"] = f2h
    d["wf2T_lo"] = (f2T - f2h.astype(f32)).astype(bf)
    d["onesr"] = np.ones((128, 1), f32)
    d["onesrw"] = np.ones((2, 128), f32)

    def cb(v, nct):
        return np.ascontiguousarray(np.asarray(v, f32).reshape(nct, 128).T)

    d["gq"], d["bq"] = cb(inp["qg"], CT_C), cb(inp["qbeta"], CT_C)
    d["gk"], d["bk"] = cb(inp["kg"], CT_C), cb(inp["kbeta"], CT_C)
    d["gp"], d["bp"] = cb(inp["pg"], CT_C), cb(inp["pbeta"], CT_C)
    d["gf1"], d["bf1"] = cb(inp["f1g"], CT_H), cb(inp["f1beta"], CT_H)
    d["gf2"], d["bf2"] = cb(inp["f2g"], CT_C), cb(inp["f2beta"], CT_C)
    # v BN affine with SCALE folded; [1, C] rows
    d["gv"] = np.ascontiguousarray(
        (np.asarray(inp["vg"], f32) * SCALE).reshape(1, C))
    d["bv"] = np.ascontiguousarray(
        (np.asarray(inp["vbeta"], f32) * SCALE).reshape(1, C))
    return d


def prep_in_maps(inputs):
    x = np.asarray(inputs["x"], np.float32)
    shared = _prep_shared(inputs)
    in_maps = []
    for core in range(NCORES):
        b0 = core * BL
        xT = np.ascontiguousarray(
            x[:, b0:b0 + BL].transpose(3, 0, 1, 2).reshape(C, R))
        m = dict(shared)
        m["xT"] = xT
        in_maps.append(m)
    return in_maps


def assemble_output(results):
    out = np.empty((T, B, N, C), np.float32)
    for core in range(NCORES):
        b0 = core * BL
        oc = np.asarray(results[core]["outT"])
        out[:, b0:b0 + BL] = oc.reshape(C, T, BL, N).transpose(1, 2, 3, 0)
    return out


def kernel(**inputs):
    from concourse.bass_utils import run_bass_kernel_spmd
    nc = build_nc()
    in_maps = prep_in_maps(inputs)
    res = run_bass_kernel_spmd(nc, in_maps, core_ids=list(range(NCORES)))
    return assemble_output(res.results)


if __name__ == "__main__":
    build_nc()
    print("build + compile OK")


# revision 35
# speedup vs baseline: 1.0233x; 1.0233x over previous
"""Trainium2 Bass kernel for nn_Block_65841848648273 (spiking transformer block).

Sharding: data-parallel over B (16 -> 2 per core x 8 cores), sync-BN via
AllReduce of per-channel (sum, sumsq). Activations kept column-major
[C, T*B_loc*N] so BN stats are per-partition free-dim reductions and LIF
runs on free-dim slices. v is computed row-major [R, C] directly (swapped
matmul operands) for the attention o = attn @ v contraction.

All dense fp32 GEMMs (q/k/v/proj/fc1 and attn@v) run in float32r: the PE
streams fp32 operands at bf16 row rate (1 cyc/row for free dim >= 256)
with ~2^-13 relative rounding — measured 1.5e-4 matmul rel err on HW,
well inside the 2e-2 gate. Spikes are exact 0/1 in bf16, so attention
q@k^T runs at bf16 rate and fc2 runs as two bf16 passes against
hi/lo-split weights with the h spikes kept resident in SBUF (no DRAM
spill round-trip).

Engine schedule: matmul phases are emitted k -> v -> q so the v-stats
AllReduce and its affine-coefficient chain hide under the q GEMM + q LIF;
the attention QK stage runs one block ahead of AV; fc1 pipelines 3 y1
chunks deep; fc2 accumulates kt-outer so it overlaps the fc1 tail. LIF
state updates (w += 2^t z, hard reset w *= (s < 0.5)) run as fused
scalar_tensor_tensor ops on the otherwise-idle Pool engine, interleaved
across ctiles so no engine stalls on one chain.

LIF rescaling: v_t = (v_{t-1} + z_t)/2  ==>  w_t = w_{t-1} + 2^t z_t with
w_t = 2^{t+1} v_t; spike iff w_t >= 2^{t+1}; hard reset on spike.
Linear biases are dropped entirely: BN(y + b) == BN(y) exactly.
The SSA 0.125 scale is folded into the v-layer BN affine on the host.
"""

import numpy as np
import ml_dtypes

import concourse.bass as bass
import concourse.mybir as mybir
import concourse.tile as tile
from concourse import bacc
from concourse.bass import ts, ds

F32 = mybir.dt.float32
F32R = mybir.dt.float32r
BF16 = mybir.dt.bfloat16
FP8 = mybir.dt.float8e4
ALU = mybir.AluOpType
ACTF = mybir.ActivationFunctionType

T, B, N, C, H, HID = 4, 16, 256, 512, 8, 2048
NCORES = 8
BL = B // NCORES            # 2 batches per core
R = T * BL * N              # 2048 rows per core
TS_ = BL * N                # 512 rows per timestep
DH = C // H                 # 64
EPS = 1e-5
SCALE = 0.125
NTOT = T * B * N            # 16384 global BN count

CT_C = C // 128             # 4
CT_H = HID // 128            # 16
RC = R // 512               # 4 row chunks
RT = R // 128               # 16 row tiles

_BUILD_CACHE = {}


def build_nc(single_core=False):
    key = "nc1" if single_core else "nc"
    if key in _BUILD_CACHE:
        return _BUILD_CACHE[key]
    nc = bacc.Bacc("TRN2", target_bir_lowering=False, debug=False,
                   enable_asserts=True,
                   num_devices=1 if single_core else NCORES)

    tens = {}
    tens["xT"] = nc.dram_tensor("xT", [C, R], F32R, kind="ExternalInput")
    for nm, shp, dt_ in (("wqT", [C, C], F32R), ("wkT", [C, C], F32R),
                         ("wvT", [C, C], F32R), ("wpT", [C, C], F32R),
                         ("wf1T", [C, HID], F32R),
                         ("wf2T_hi", [HID, C], BF16), ("wf2T_lo", [HID, C], BF16)):
        tens[nm] = nc.dram_tensor(nm, shp, dt_, kind="ExternalInput")
    tens["onesr"] = nc.dram_tensor("onesr", [128, 2], F32R, kind="ExternalInput")
    for nm, nct in (("gq", CT_C), ("bq", CT_C), ("gk", CT_C), ("bk", CT_C),
                    ("gp", CT_C), ("bp", CT_C), ("gf1", CT_H), ("bf1", CT_H),
                    ("gf2", CT_C), ("bf2", CT_C)):
        tens[nm] = nc.dram_tensor(nm, [128, nct], F32, kind="ExternalInput")
    # v-layer gamma/beta as [1, C] rows, SCALE folded
    tens["gv"] = nc.dram_tensor("gv", [1, C], F32, kind="ExternalInput")
    tens["bv"] = nc.dram_tensor("bv", [1, C], F32, kind="ExternalInput")
    tens["outT"] = nc.dram_tensor("outT", [C, R], F32R, kind="ExternalOutput")

    with tile.TileContext(nc, pool_alloc_mode="queue") as tc:
        _emit(nc, tc, tens, single_core=single_core)
    nc.compile()
    _BUILD_CACHE[key] = nc
    return nc


def _emit(nc, tc, tens, single_core=False):
    groups = [list(range(NCORES))]

    def all_reduce(arin, arout):
        # DRAM in/out: SBUF collectives are broken on current HW/runtime
        if single_core:
            # timing-sim variant: no collectives; a DRAM->DRAM copy via the
            # SP DMA queue stands in (real AR latency is modeled separately)
            nc.sync.dma_start(out=arout[:], in_=arin[:])
        else:
            nc.gpsimd.collective_compute(
                "AllReduce", ALU.add, replica_groups=groups,
                ins=[arin.opt()], outs=[arout.opt()])

    # ---------- long-lived pools ----------
    small = tc.alloc_tile_pool(name="small", bufs=1)
    stats = tc.alloc_tile_pool(name="stats", bufs=1)
    dram = tc.alloc_tile_pool(name="dram", bufs=1, space="DRAM")

    ones_col = small.tile([128, 2], F32R)
    nc.sync.dma_start(out=ones_col, in_=tens["onesr"].ap())
    gb = {}
    for nm in ("gq", "bq", "gk", "bk", "gp", "bp", "gf1", "bf1", "gf2", "bf2"):
        tl = small.tile(list(tens[nm].shape), F32, name=f"sb_{nm}")
        nc.sync.dma_start(out=tl, in_=tens[nm].ap())
        gb[nm] = tl

    def alloc_coeffs(name, nct):
        return {"a": small.tile([128, nct], F32, name=f"{name}_a"),
                "c": small.tile([128, nct], F32, name=f"{name}_c"),
                "at": small.tile([128, T, nct], F32, name=f"{name}_at"),
                "ct": small.tile([128, T, nct], F32, name=f"{name}_ct")}

    cf_k = alloc_coeffs("k", CT_C)
    cf_q = alloc_coeffs("q", CT_C)
    cf_p = alloc_coeffs("p", CT_C)
    cf_f1 = [alloc_coeffs(f"f1_{j}", 2) for j in range(8)]
    cf_f2 = alloc_coeffs("f2", CT_C)

    # ---------- weights + x ----------
    px = tc.alloc_tile_pool(name="px", bufs=1)
    x_sb = px.tile([128, CT_C, R], F32R)
    # one pool whose 2 slots rotate through all the 32KB/partition tensors
    pyy = tc.alloc_tile_pool(name="pyy", bufs=3)
    pw = tc.alloc_tile_pool(name="pw", bufs=2)
    w_sb = {}
    for nm in ("wkT", "wvT", "wqT", "wpT"):
        w_sb[nm] = pw.tile([128, CT_C, C], F32R, name=f"sb_{nm}", tag="w")
    _xr = tens["xT"].ap().rearrange("(ci p) r -> p ci r", p=128)
    _wr = {nm: tens[nm].ap().rearrange("(ci p) co -> p ci co", p=128)
           for nm in ("wkT", "wqT", "wvT", "wpT")}
    # first matmul group needs wk + x[:, :, rc0]: land those first
    nc.sync.dma_start(out=w_sb["wkT"], in_=_wr["wkT"])
    for rc in range(RC):
        for ci in range(CT_C):
            eng = nc.sync if (ci % 2 == 0) else nc.scalar
            eng.dma_start(out=x_sb[:, ci, ts(rc, 512)],
                          in_=_xr[:, ci, ts(rc, 512)])
        if rc == 0:
            nc.sync.dma_start(out=w_sb["wvT"], in_=_wr["wvT"])
    pspk = tc.alloc_tile_pool(name="pspk", bufs=1)
    k_spk = pspk.tile([128, CT_C, R], FP8)
    q_spk = pspk.tile([128, CT_C, R], FP8)

    ps1 = tc.alloc_tile_pool(name="ps1", bufs=6, space="PSUM")

    # ================= helpers =================
    def mm_col(wtl, dst, rhs, nm, pspool, pstag, couts=range(CT_C),
               post_tile=None, post_co=None, evict_split=False):
        for co in couts:
            for rc in range(RC):
                ps = pspool.tile([128, 512], F32, tag=pstag,
                                 name=f"ps_{nm}_{co}_{rc}")
                for ci in range(CT_C):
                    nc.tensor.matmul(ps, wtl[:, ci, ts(co, 128)],
                                     rhs[:, ci, ts(rc, 512)],
                                     start=(ci == 0), stop=(ci == CT_C - 1))
                if evict_split and rc % 2 == 1:
                    nc.vector.tensor_copy(dst[:, co, ts(rc, 512)], ps)
                else:
                    nc.scalar.copy(dst[:, co, ts(rc, 512)], ps)
                if post_tile is not None:
                    post_tile(co, rc)
            if post_co is not None:
                post_co(co)

    def make_packer(nct, nm):
        """Interleavable bn_stats: bnst(ct, rc, y_rc) per evicted tile,
        aggr(ct) once a ctile is complete, finish() -> AllReduce handle."""
        bn = stats.tile([128, nct, RC, 6], F32, name=f"bn_{nm}", tag="bn6",
                        bufs=2)
        mv = stats.tile([128, nct, 2], F32, name=f"mv_{nm}", tag="mv")

        def bnst(ct, rc, y_rc):
            nc.vector.bn_stats(out=bn[:, ct, rc, :], in_=y_rc)

        def aggr(ct):
            nc.vector.bn_aggr(out=mv[:, ct, :], in_=bn[:, ct, :, :])

        def finish():
            pk = stats.tile([128, nct, 2], F32, name=f"pk_{nm}", tag="pk")
            m2 = stats.tile([128, nct], F32, name=f"m2_{nm}", tag="m2s")
            nc.vector.tensor_scalar(out=pk[:, :, 0:1], in0=mv[:, :, 0:1],
                                    scalar1=float(R), scalar2=None, op0=ALU.mult)
            nc.vector.tensor_tensor(out=m2, in0=mv[:, :, 0:1], in1=mv[:, :, 0:1],
                                    op=ALU.mult)
            nc.vector.tensor_tensor(out=m2, in0=mv[:, :, 1:2], in1=m2, op=ALU.add)
            nc.vector.tensor_scalar(out=pk[:, :, 1:2], in0=m2, scalar1=float(R),
                                    scalar2=None, op0=ALU.mult)
            arin = dram.tile([128, nct, 2], F32, name=f"arin_{nm}")
            arout = dram.tile([128, nct, 2], F32, name=f"arout_{nm}")
            nc.sync.dma_start(out=arin, in_=pk)
            all_reduce(arin, arout)
            return arout

        return bnst, aggr, finish

    def pack_hooks(dst, nct, nm):
        bnst, aggr, finish = make_packer(nct, nm)

        def post_tile(co, rc):
            bnst(co, rc, dst[:, co, ts(rc, 512)])

        def post_co(co):
            aggr(co)

        return post_tile, post_co, finish

    def coeffs(arout, nct, g_tl, b_tl, cfd, nm, at_off=0):
        """AR result -> a = g*rstd, c = beta - a*mean."""
        ar_sb = stats.tile([128, nct, 2], F32, name=f"ars_{nm}", tag="ars",
                           bufs=2)
        nc.sync.dma_start(out=ar_sb, in_=arout[:])
        mean = stats.tile([128, nct], F32, name=f"mea_{nm}", tag="mea")
        var = stats.tile([128, nct], F32, name=f"var_{nm}", tag="vars")
        m2 = stats.tile([128, nct], F32, name=f"m2c_{nm}", tag="m2c")
        nc.vector.tensor_scalar(out=mean, in0=ar_sb[:, :, 0:1], scalar1=1.0 / NTOT,
                                scalar2=None, op0=ALU.mult)
        nc.vector.tensor_scalar(out=var, in0=ar_sb[:, :, 1:2], scalar1=1.0 / NTOT,
                                scalar2=None, op0=ALU.mult)
        nc.vector.tensor_tensor(out=m2, in0=mean, in1=mean, op=ALU.mult)
        nc.vector.tensor_tensor(out=var, in0=var, in1=m2, op=ALU.subtract)
        # u = var + eps; r0 ~ rsqrt(u) via LUT sqrt + reciprocal, then one
        # Newton step r1 = r0*(1.5 - 0.5*u*r0^2) to reach fp32 accuracy
        # (ACT Sqrt LUT alone is ~7e-6 worst-case -> flips LIF spikes).
        u = stats.tile([128, nct], F32, name=f"u_{nm}", tag="rsu")
        nc.vector.tensor_scalar(out=u, in0=var, scalar1=EPS, scalar2=None,
                                op0=ALU.add)
        nc.scalar.activation(out=var, in_=u, func=ACTF.Sqrt)
        nc.vector.reciprocal(out=var, in_=var)          # r0
        nt = stats.tile([128, nct], F32, name=f"nt_{nm}", tag="rsn")
        nc.vector.tensor_tensor(out=nt, in0=u, in1=var, op=ALU.mult)
        nc.vector.tensor_tensor(out=nt, in0=nt, in1=var, op=ALU.mult)
        nc.vector.tensor_scalar(out=nt, in0=nt, scalar1=-0.5, scalar2=1.5,
                                op0=ALU.mult, op1=ALU.add)
        nc.vector.tensor_tensor(out=var, in0=var, in1=nt, op=ALU.mult)  # rstd
        a_dst = cfd["a"] if at_off == 0 and cfd["a"].shape[1] == nct \
            else cfd["a"][:, at_off:at_off + nct]
        c_dst = cfd["c"] if at_off == 0 and cfd["c"].shape[1] == nct \
            else cfd["c"][:, at_off:at_off + nct]
        nc.vector.tensor_tensor(out=a_dst, in0=g_tl, in1=var, op=ALU.mult)
        nc.vector.tensor_tensor(out=m2, in0=a_dst, in1=mean, op=ALU.mult)
        nc.vector.tensor_tensor(out=c_dst, in0=b_tl, in1=m2, op=ALU.subtract)
        for t in range(T):
            sc = float(2 ** t)
            nc.vector.tensor_scalar(out=cfd["at"][:, t, at_off:at_off + nct],
                                    in0=a_dst, scalar1=sc, scalar2=None,
                                    op0=ALU.mult)
            nc.vector.tensor_scalar(out=cfd["ct"][:, t, at_off:at_off + nct],
                                    in0=c_dst, scalar1=sc, scalar2=None,
                                    op0=ALU.mult)

    def lif_multi(items, nm, z_dve=False, add_dve=False, post_t=None):
        """Interleaved LIF over 2-ctile-wide groups: z = a*y + c per ctile
        on Act (or DVE two-scalar form when z_dve); the 2^t scale folds
        into a single wide Pool stt accumulate; reset is one fused wide
        stt w *= (s < 0.5); threshold compare is one wide DVE op.

        items: list of (ywide [128,G,R], cfd, c0, G, spk(t)->[128,G,TS_],
        extra(t, s_ap)|None).
        """
        ws = [stats.tile([128, it[3], TS_], F32, name=f"lw_{nm}_{i}",
                         tag="lifw", bufs=2) for i, it in enumerate(items)]
        for t in range(T):
            sl = ts(t, TS_)
            zs = []
            for i, (yw, cfd, c0, G, spk, extra) in enumerate(items):
                zdst = ws[i] if t == 0 else stats.tile(
                    [128, G, TS_], F32, name=f"lz_{nm}_{i}_{t}", tag="lifz",
                    bufs=2)
                for g in range(G):
                    cg = c0 + g
                    if z_dve:
                        nc.vector.tensor_scalar(out=zdst[:, g, :],
                                                in0=yw[:, g, sl],
                                                scalar1=cfd["at"][:, t, cg:cg + 1],
                                                scalar2=cfd["ct"][:, t, cg:cg + 1],
                                                op0=ALU.mult, op1=ALU.add)
                    else:
                        nc.scalar.activation(out=zdst[:, g, :], in_=yw[:, g, sl],
                                             func=ACTF.Identity,
                                             bias=cfd["ct"][:, t, cg:cg + 1],
                                             scale=cfd["at"][:, t, cg:cg + 1])
                zs.append(zdst)
            if t > 0:
                for i in range(len(items)):
                    if add_dve:
                        nc.vector.tensor_tensor(out=ws[i], in0=zs[i],
                                                in1=ws[i], op=ALU.add)
                    else:
                        nc.gpsimd.tensor_tensor(out=ws[i], in0=zs[i],
                                                in1=ws[i], op=ALU.add)
            s_aps = []
            for i, (yw, cfd, c0, G, spk, extra) in enumerate(items):
                s_ap = spk(t)
                nc.vector.tensor_scalar(out=s_ap, in0=ws[i],
                                        scalar1=float(2 ** (t + 1)),
                                        scalar2=None, op0=ALU.is_ge)
                s_aps.append(s_ap)
            for i, (yw, cfd, c0, G, spk, extra) in enumerate(items):
                if t < T - 1:
                    # hard reset w *= (s < 0.5): fused stt, DVE only (the
                    # Pool engine has no TensorScalar-class ISA support)
                    nc.vector.scalar_tensor_tensor(out=ws[i], in0=s_aps[i],
                                                   scalar=0.5, in1=ws[i],
                                                   op0=ALU.is_lt, op1=ALU.mult)
                if extra is not None:
                    extra(t, s_aps[i])
            if post_t is not None:
                post_t(t)

    # ============ phase 1+2: k GEMM -> v GEMM (+stats AR) -> q GEMM ======
    y_k = pyy.tile([128, CT_C, R], F32, tag="yy", name="y_k")
    pt_k, pc_k, fin_k = pack_hooks(y_k, CT_C, "k")
    mm_col(w_sb["wkT"], y_k, x_sb, "k", ps1, "mmps", post_tile=pt_k,
           post_co=pc_k)
    ar_k = fin_k()

    # v row-major, float32r; channel sums ride the PE via ones-matmuls
    v_sb = pyy.tile([128, RT, C], F32R, tag="yy", name="v_sb")
    pvsq = tc.alloc_tile_pool(name="pvsq", bufs=1)
    for nm in ("gv", "bv"):
        tl = pvsq.tile(list(tens[nm].shape), F32, name=f"sb_{nm}")
        nc.sync.dma_start(out=tl, in_=tens[nm].ap())
        gb[nm] = tl
    psv = tc.alloc_tile_pool(name="psv", bufs=1, space="PSUM")
    vsum_ps = psv.tile([2, C], F32)
    vsq_ps = psv.tile([2, C], F32)
    for rt in range(RT):
        ps = ps1.tile([128, 512], F32, tag="mmps", name=f"ps_v_{rt}")
        for ci in range(CT_C):
            nc.tensor.matmul(ps, x_sb[:, ci, ts(rt, 128)], w_sb["wvT"][:, ci, :],
                             start=(ci == 0), stop=(ci == CT_C - 1))
        if rt % 2 == 0:
            nc.scalar.copy(v_sb[:, rt, :], ps)
        else:
            nc.vector.tensor_copy(v_sb[:, rt, :], ps)
        vsq = pvsq.tile([128, C], F32R, name=f"vsq_{rt}", tag="vsq", bufs=2)
        nc.vector.scalar_tensor_tensor(out=vsq, in0=v_sb[:, rt, :], scalar=1.0,
                                       in1=v_sb[:, rt, :], op0=ALU.mult,
                                       op1=ALU.mult)
        nc.tensor.matmul(vsum_ps, ones_col, v_sb[:, rt, :],
                         start=(rt == 0), stop=(rt == RT - 1), skip_group_check=True)
        nc.tensor.matmul(vsq_ps, ones_col, vsq,
                         start=(rt == 0), stop=(rt == RT - 1), skip_group_check=True)

    # v stats AllReduce kicked off immediately (hides under q GEMM + LIFs)
    arin_v = dram.tile([2, C], F32)
    arout_v = dram.tile([2, C], F32)
    vsum_sb1 = pvsq.tile([1, C], F32, name="vsum_sb1")
    vsq_sb1 = pvsq.tile([1, C], F32, name="vsq_sb1")
    nc.scalar.copy(vsum_sb1, vsum_ps[0:1, :])
    nc.scalar.copy(vsq_sb1, vsq_ps[0:1, :])
    nc.sync.dma_start(out=arin_v[0:1, :], in_=vsum_sb1)
    nc.sync.dma_start(out=arin_v[1:2, :], in_=vsq_sb1)
    all_reduce(arin_v, arout_v)

    nc.sync.dma_start(out=w_sb["wpT"], in_=_wr["wpT"])
    coeffs(ar_k, CT_C, gb["gk"], gb["bk"], cf_k, "k")
    lif_multi([(y_k[:, 2 * i:2 * i + 2, :], cf_k, 2 * i, 2,
                (lambda t, i=i: k_spk[:, 2 * i:2 * i + 2, ts(t, TS_)]), None)
               for i in range(2)], "k")

    nc.sync.dma_start(out=w_sb["wqT"], in_=_wr["wqT"])
    y_q = pyy.tile([128, CT_C, R], F32, tag="yy", name="y_q")
    pt_q, pc_q, fin_q = pack_hooks(y_q, CT_C, "q")
    mm_col(w_sb["wqT"], y_q, x_sb, "q", ps1, "mmps", post_tile=pt_q,
           post_co=pc_q)
    ar_q = fin_q()

    # ---- v affine in [1, C] row layout ----
    # BN(v) = a*(v+c/a) with a folded into the proj weights (row scale), so
    # only one elementwise add per v tile remains.
    _arv = arout_v[:]
    nc.sync.dma_start(out=vsum_sb1, in_=bass.AP(
        tensor=_arv.tensor, offset=_arv.offset, ap=[[C, 1], [1, C]]))
    nc.sync.dma_start(out=vsq_sb1, in_=bass.AP(
        tensor=_arv.tensor, offset=_arv.offset + C, ap=[[C, 1], [1, C]]))
    vmean = pvsq.tile([1, C], F32, name="vmean")
    vvar = pvsq.tile([1, C], F32, name="vvar")
    vm2 = pvsq.tile([1, C], F32, name="vm2")
    va_row = vsum_sb1
    vc_row = vsq_sb1
    nc.vector.tensor_scalar(out=vmean, in0=vsum_sb1, scalar1=1.0 / NTOT,
                            scalar2=None, op0=ALU.mult)
    nc.vector.tensor_scalar(out=vvar, in0=vsq_sb1, scalar1=1.0 / NTOT,
                            scalar2=None, op0=ALU.mult)
    nc.vector.tensor_tensor(out=vm2, in0=vmean, in1=vmean, op=ALU.mult)
    nc.vector.tensor_tensor(out=vvar, in0=vvar, in1=vm2, op=ALU.subtract)
    vnt = pvsq.tile([1, C], F32, name="v_nt")
    nc.vector.tensor_scalar(out=vm2, in0=vvar, scalar1=EPS, scalar2=None,
                            op0=ALU.add)                       # vm2 = var+eps
    nc.scalar.activation(out=vvar, in_=vm2, func=ACTF.Sqrt)
    nc.vector.reciprocal(out=vvar, in_=vvar)
    nc.vector.tensor_tensor(out=vnt, in0=vm2, in1=vvar, op=ALU.mult)
    nc.vector.tensor_tensor(out=vnt, in0=vnt, in1=vvar, op=ALU.mult)
    nc.vector.tensor_scalar(out=vnt, in0=vnt, scalar1=-0.5, scalar2=1.5,
                            op0=ALU.mult, op1=ALU.add)
    nc.vector.tensor_tensor(out=vvar, in0=vvar, in1=vnt, op=ALU.mult)  # rstd
    nc.vector.tensor_tensor(out=va_row, in0=gb["gv"], in1=vvar, op=ALU.mult)
    nc.vector.tensor_tensor(out=vm2, in0=va_row, in1=vmean, op=ALU.mult)
    nc.vector.tensor_tensor(out=vc_row, in0=gb["bv"], in1=vm2, op=ALU.subtract)
    # cp = c/a; broadcast to 128 partitions via a K=1 PE matmul
    nc.vector.reciprocal(out=vnt, in_=va_row)
    cp_row = pvsq.tile([1, C], F32, name="cp_row")
    nc.vector.tensor_tensor(out=cp_row, in0=vc_row, in1=vnt, op=ALU.mult)
    cp_dram = dram.tile([1, C], F32, name="cp_dram")
    nc.sync.dma_start(out=cp_dram, in_=cp_row)
    cp_bc = pspk.tile([128, C], F32, name="cp_bc")
    nc.gpsimd.dma_start(out=cp_bc,
                        in_=cp_dram[:].flatten().partition_broadcast(128))
    # scale the proj weight rows by a_d in place (a in [128, CT_C] ci-major)
    a_dram = dram.tile([1, C], F32, name="a_dram")
    nc.sync.dma_start(out=a_dram, in_=va_row)
    a_ci = stats.tile([128, CT_C], F32, name="a_ci")
    _ad = a_dram[:]
    nc.sync.dma_start(out=a_ci, in_=bass.AP(
        tensor=_ad.tensor, offset=_ad.offset, ap=[[1, 128], [128, CT_C]]))
    for ci in range(CT_C):
        nc.vector.tensor_scalar(out=w_sb["wpT"][:, ci, :],
                                in0=w_sb["wpT"][:, ci, :],
                                scalar1=a_ci[:, ci:ci + 1], scalar2=None,
                                op0=ALU.mult)

    coeffs(ar_q, CT_C, gb["gq"], gb["bq"], cf_q, "q")
    pvsq.release()
    psv.release()
    ps1.release()

    # ======= phase 3: attention, interleaved with the q LIF rounds =======
    # QK for blocks (2t, 2t+1) is emitted right after LIF round t produces
    # the t-slice of q spikes; AV trails one block behind its QK.
    oT = pyy.tile([128, CT_C, R], F32R, tag="yy", name="oT")
    ps_at = tc.alloc_tile_pool(name="ps_at", bufs=3, space="PSUM")
    ps_o = tc.alloc_tile_pool(name="ps_o", bufs=3, space="PSUM")
    p_at = tc.alloc_tile_pool(name="p_at", bufs=9)

    def emit_qk(tb):
        t, b = divmod(tb, BL)
        row0 = t * TS_ + b * N
        tiles = {}
        for h in range(H):
            hp = 64 * (h % 2)
            kq_p = slice(hp, hp + 64)
            hc = h // 2
            atp = ps_at.tile([128, 2, 256], F32, tag="at",
                             name=f"atp_{tb}_{h}")
            for mt in range(2):
                nc.tensor.matmul(atp[:, mt, :],
                                 k_spk[kq_p, hc, ds(row0 + mt * 128, 128)],
                                 q_spk[kq_p, hc, ds(row0, 256)],
                                 start=True, stop=True)
            at_sb = p_at.tile([128, 2, 256], F32R, tag="atsb",
                              name=f"at_{tb}_{h}")
            # split PSUM evictions across Act and DVE
            if h % 2 == 0:
                nc.scalar.copy(at_sb, atp)
            else:
                nc.vector.tensor_copy(at_sb, atp)
            tiles[h] = at_sb
        return tiles

    def emit_av(tb, tiles):
        t, b = divmod(tb, BL)
        row0 = t * TS_ + b * N
        rt0 = tb * 2
        for hp_i in range(H // 2):
            op = ps_o.tile([128, 256], F32, tag="o", name=f"op_{tb}_{hp_i}")
            for sub in range(2):
                h = hp_i * 2 + sub
                hp = 64 * sub
                for mt in range(2):
                    nc.tensor.matmul(op[hp:hp + 64, :],
                                     v_sb[:, rt0 + mt, ds(h * DH, DH)],
                                     tiles[h][:, mt, :],
                                     start=(mt == 0), stop=(mt == 1),
                                     tile_position=(0, hp))
            if hp_i % 2 == 0:
                nc.scalar.copy(oT[:, hp_i, ds(row0, 256)], op)
            else:
                nc.vector.tensor_copy(oT[:, hp_i, ds(row0, 256)], op)

    ats = {}

    def attn_post(t):
        # v + c/a for the row tiles this round's AV blocks consume (Pool)
        for rt in range(4 * t, 4 * t + 4):
            nc.gpsimd.tensor_tensor(out=v_sb[:, rt, :], in0=v_sb[:, rt, :],
                                    in1=cp_bc, op=ALU.add)
        for tb in (2 * t, 2 * t + 1):
            ats[tb] = emit_qk(tb)
            if tb - 1 in ats:
                emit_av(tb - 1, ats.pop(tb - 1))

    lif_multi([(y_q[:, 2 * i:2 * i + 2, :], cf_q, 2 * i, 2,
                (lambda t, i=i: q_spk[:, 2 * i:2 * i + 2, ts(t, TS_)]), None)
               for i in range(2)], "q", z_dve=True, post_t=attn_post)
    emit_av(T * BL - 1, ats.pop(T * BL - 1))

    ps_o.release()
    ps_at.release()
    p_at.release()
    pspk.release()

    # ================= phase 4: proj + residual (x2 in place on x_sb) =====
    ps2 = tc.alloc_tile_pool(name="ps2", bufs=4, space="PSUM")
    y_p = pyy.tile([128, CT_C, R], F32, tag="yy", name="y_p")
    pt_p, pc_p, fin_p = pack_hooks(y_p, CT_C, "p")
    mm_col(w_sb["wpT"], y_p, oT, "p", ps2, "mmps2", post_tile=pt_p,
           post_co=pc_p)

    # fc1 (f32r) weights loaded before the p-stats AllReduce is queued, so
    # the waiting AR copy can't block them on the SP DMA queue. The same
    # 8KB slots later rotate to the fc2 bf16 hi/lo weights.
    pw.release()
    pf = tc.alloc_tile_pool(name="pf", bufs=5, side="right")
    f1w = []
    for ci in range(CT_C):
        tw = pf.tile([128, HID], F32R, name=f"f1w_{ci}", tag="wbig")
        nc.sync.dma_start(out=tw, in_=tens["wf1T"].ap()[ts(ci, 128), :])
        f1w.append(tw)
    ar_p = fin_p()

    coeffs(ar_p, CT_C, gb["gp"], gb["bp"], cf_p, "p")
    for rc in range(RC):
        for ct in range(CT_C):
            z = stats.tile([128, 512], F32, name=f"pz_{ct}_{rc}", tag="lifz",
                           bufs=2)
            nc.vector.tensor_scalar(out=z, in0=y_p[:, ct, ts(rc, 512)],
                                    scalar1=cf_p["a"][:, ct:ct + 1],
                                    scalar2=cf_p["c"][:, ct:ct + 1],
                                    op0=ALU.mult, op1=ALU.add)
            nc.gpsimd.tensor_tensor(out=x_sb[:, ct, ts(rc, 512)],
                                    in0=x_sb[:, ct, ts(rc, 512)], in1=z,
                                    op=ALU.add)

    # ================= phase 5: fc1 + LIF (single float32r pass) ==========
    pyy.release()
    # h spikes stay resident in SBUF (64KB/partition, bf16 0/1 exact)
    ph_sb = tc.alloc_tile_pool(name="ph_sb", bufs=1, side="right")
    h_sb = ph_sb.tile([128, CT_H, R], BF16, name="h_sb")
    py1 = tc.alloc_tile_pool(name="py1", bufs=2, side="right")
    f1_chunks = [(c0, 2) for c0 in range(0, CT_H, 2)]
    for j, (co0, ncts) in enumerate(f1_chunks):
        y1 = py1.tile([128, ncts, R], F32, tag="yy1", name=f"y1_{j}")
        bnst, aggr, fin = make_packer(ncts, f"f1_{j}")
        for c4 in range(ncts):
            co = co0 + c4
            for rc in range(RC):
                ps = ps2.tile([128, 512], F32, tag="mmps2", name=f"ps_f1_{co}_{rc}")
                for ci in range(CT_C):
                    nc.tensor.matmul(ps, f1w[ci][:, ts(co, 128)],
                                     x_sb[:, ci, ts(rc, 512)],
                                     start=(ci == 0), stop=(ci == CT_C - 1))
                nc.scalar.copy(y1[:, c4, ts(rc, 512)], ps)
                bnst(c4, rc, y1[:, c4, ts(rc, 512)])
            aggr(c4)
        arout = fin()
        coeffs(arout, ncts, gb["gf1"][:, ds(co0, ncts)], gb["bf1"][:, ds(co0, ncts)],
               cf_f1[j], f"f1_{j}")
        lif_multi([(y1, cf_f1[j], 0, 2,
                    (lambda t, co0=co0: h_sb[:, co0:co0 + 2, ts(t, TS_)]),
                    None)], f"h{co0}")

    # fc2 bf16 hi/lo weights, split by co-half so the first fc2 quarter
    # only waits on a quarter of the load bytes
    f2w = {}
    for coh in range(2):
        for wi, src in enumerate(("wf2T_hi", "wf2T_lo")):
            tw = pf.tile([128, CT_H, 256], BF16, name=f"f2w_{wi}_{coh}",
                         tag="wbig")
            nc.sync.dma_start(
                out=tw,
                in_=tens[src].ap()[:, ts(coh, 256)].rearrange(
                    "(kt p) co -> p kt co", p=128))
            f2w[(wi, coh)] = tw

    # ================= phase 6: fc2 + LIF + out =================
    py1.release()
    ps2.release()
    py2 = tc.alloc_tile_pool(name="py2", bufs=2, side="right")
    ps3 = tc.alloc_tile_pool(name="ps3", bufs=8, space="PSUM")
    for half in range(2):
        cos = (half * 2, half * 2 + 1)
        y2 = py2.tile([128, 2, R], F32, tag="yy2", name=f"y2_{half}")
        bnst2, aggr2, fin2 = make_packer(2, f"f2_{half}")
        # kt-outer accumulation in 4-psum quarters: early-kt matmuls only
        # depend on early fc1 chunks, so fc2 overlaps the fc1 tail
        for rcp in range(2):
            rcs = (rcp * 2, rcp * 2 + 1)
            pss = {}
            for co in cos:
                for rc in rcs:
                    pss[(co, rc)] = ps3.tile([128, 512], F32, tag="mm3",
                                             name=f"ps_f2_{co}_{rc}")
            for kt in range(CT_H):
                for wi in range(2):
                    for co in cos:
                        wsb = f2w[(wi, co // 2)]
                        for rc in rcs:
                            nc.tensor.matmul(
                                pss[(co, rc)], wsb[:, kt, ts(co % 2, 128)],
                                h_sb[:, kt, ts(rc, 512)],
                                start=(kt == 0 and wi == 0),
                                stop=(kt == CT_H - 1 and wi == 1))
            for ic, co in enumerate(cos):
                for rc in rcs:
                    nc.scalar.copy(y2[:, ic, ts(rc, 512)], pss[(co, rc)])
                    bnst2(ic, rc, y2[:, ic, ts(rc, 512)])
                    if rcp == 1 and rc == rcs[-1]:
                        aggr2(ic)
        arout = fin2()
        coeffs(arout, 2, gb["gf2"][:, ts(half, 2)], gb["bf2"][:, ts(half, 2)],
               cf_f2, f"f2_{half}", at_off=half * 2)

        def spk_ap2(t):
            stile = stats.tile([128, 2, TS_], BF16, name=f"ms_{half}_{t}",
                               tag="mspk", bufs=1)
            return stile[:]

        def resadd(t, s_ap, c0=cos[0]):
            sl = ts(t, TS_)
            nc.vector.tensor_tensor(out=x_sb[:, c0:c0 + 2, sl],
                                    in0=x_sb[:, c0:c0 + 2, sl],
                                    in1=s_ap, op=ALU.add)

        def resout(t, s_ap, c0=cos[0]):
            resadd(t, s_ap, c0)
            sl = ts(t, TS_)
            for co in cos:
                nc.sync.dma_start(out=tens["outT"].ap()[ts(co, 128), sl],
                                  in_=x_sb[:, co, sl])

        lif_multi([(y2, cf_f2, half * 2, 2, spk_ap2, resout)],
                  f"m{half}", add_dve=True)

    ps3.release()
    py2.release()
    ph_sb.release()
    pf.release()
    px.release()
    stats.release()
    small.release()
    dram.release()


# ===================== host side =====================

def _prep_shared(inp):
    bf = ml_dtypes.bfloat16
    f32 = np.float32
    d = {}
    d["wqT"] = np.ascontiguousarray(np.asarray(inp["qw"], f32).T)
    d["wkT"] = np.ascontiguousarray(np.asarray(inp["kw"], f32).T)
    d["wvT"] = np.ascontiguousarray(np.asarray(inp["vw"], f32).T)
    d["wpT"] = np.ascontiguousarray(np.asarray(inp["pw"], f32).T)
    d["wf1T"] = np.ascontiguousarray(np.asarray(inp["f1w"], f32).T)
    f2T = np.ascontiguousarray(np.asarray(inp["f2w"], f32).T)
    f2h = f2T.astype(bf)
    d["wf2T_hi"] = f2h
    d["wf2T_lo"] = (f2T - f2h.astype(f32)).astype(bf)
    d["onesr"] = np.ones((128, 2), f32)

    def cb(v, nct):
        return np.ascontiguousarray(np.asarray(v, f32).reshape(nct, 128).T)

    d["gq"], d["bq"] = cb(inp["qg"], CT_C), cb(inp["qbeta"], CT_C)
    d["gk"], d["bk"] = cb(inp["kg"], CT_C), cb(inp["kbeta"], CT_C)
    d["gp"], d["bp"] = cb(inp["pg"], CT_C), cb(inp["pbeta"], CT_C)
    d["gf1"], d["bf1"] = cb(inp["f1g"], CT_H), cb(inp["f1beta"], CT_H)
    d["gf2"], d["bf2"] = cb(inp["f2g"], CT_C), cb(inp["f2beta"], CT_C)
    # v BN affine with SCALE folded; [1, C] rows
    d["gv"] = np.ascontiguousarray(
        (np.asarray(inp["vg"], f32) * SCALE).reshape(1, C))
    d["bv"] = np.ascontiguousarray(
        (np.asarray(inp["vbeta"], f32) * SCALE).reshape(1, C))
    return d


def prep_in_maps(inputs):
    x = np.asarray(inputs["x"], np.float32)
    shared = _prep_shared(inputs)
    in_maps = []
    for core in range(NCORES):
        b0 = core * BL
        xT = np.ascontiguousarray(
            x[:, b0:b0 + BL].transpose(3, 0, 1, 2).reshape(C, R))
        m = dict(shared)
        m["xT"] = xT
        in_maps.append(m)
    return in_maps


def assemble_output(results):
    out = np.empty((T, B, N, C), np.float32)
    for core in range(NCORES):
        b0 = core * BL
        oc = np.asarray(results[core]["outT"])
        out[:, b0:b0 + BL] = oc.reshape(C, T, BL, N).transpose(1, 2, 3, 0)
    return out


def kernel(**inputs):
    from concourse.bass_utils import run_bass_kernel_spmd
    nc = build_nc()
    in_maps = prep_in_maps(inputs)
    res = run_bass_kernel_spmd(nc, in_maps, core_ids=list(range(NCORES)))
    return assemble_output(res.results)


if __name__ == "__main__":
    build_nc()
    print("build + compile OK")
